# revision 1
# baseline (speedup 1.0000x reference)
"""AttentiveItemToVec Trainium2 kernel.

Full-input contract: kernel(**inputs) takes the unsharded numpy inputs and
returns the full [512, 101, 128] float32 output. Internally shards the batch
across 8 NeuronCores (64 batches each), runs a Bass/Tile kernel per core via
run_bass_kernel_spmd, and concatenates the per-core outputs.

Per-core (64 batches): embedding rows are fetched with multi-block indirect
DMAs (4x128 rows per instruction, padded index layout so batch b's rows land
on partitions 0..100 of block b). Per batch: PE-transpose v/u to
feature-major, project (tpT/cpT with bias), squared norms via
matmul-with-ones, 1/|x| = exp(-0.5*ln(x^2)) on ScalarE (Ln/Exp/Copy/Identity
are forced into one activation table, so the kernel pays a single table
load), cosine scores, softmax without max-subtraction (cos is in [-1,1];
pad-mask enters the exp as a -1e30 bias), attention apply, output
projection. Softmax normalization and the Bc_b/R_b biases are folded into
the output stage (attention rows sum to 1).
"""

import numpy as np
from contextlib import ExitStack

# Problem constants (hardcoded per contract).
V, E, D = 100000, 128, 60
B, J, M, P = 512, 101, 50, 5120
NCORES = 8
BLOC = B // NCORES  # 64 batches per core
NEG = -1.0e30
EPS2 = 1e-12  # clamp on squared norms (eps=1e-6 on norms)

_CACHE = {}

_ACT_TABLE = "natural_log_exp_and_others"


def _patched_tables(orig_fn):
    def fn(arch):
        tabs = orig_fn(arch)
        return {
            name: (s if name == _ACT_TABLE else type(s)())
            for name, s in tabs.items()
        }
    return fn


def _build_program():
    import os
    NOPATCH = os.environ.get("K_NOPATCH") == "1"
    import concourse.bass as bass
    import concourse.tile as tile
    import concourse.bacc as bacc_mod
    from concourse import bacc, mybir

    f32 = mybir.dt.float32
    i32 = mybir.dt.int32

    nc = bacc.Bacc(
        "TRN2",
        target_bir_lowering=False,
        debug=False,
        enable_asserts=False,
    )

    temb = nc.dram_tensor("t_emb", [V, E], f32, kind="ExternalInput").ap()
    cemb = nc.dram_tensor("c_emb", [V, E], f32, kind="ExternalInput").ap()
    atwT = nc.dram_tensor("atwT", [E, D], f32, kind="ExternalInput").ap()
    acwT = nc.dram_tensor("acwT", [E, D], f32, kind="ExternalInput").ap()
    bcwT = nc.dram_tensor("bcwT", [E, E], f32, kind="ExternalInput").ap()
    rwT = nc.dram_tensor("rwT", [E, E], f32, kind="ExternalInput").ap()
    atb = nc.dram_tensor("atb", [D, 1], f32, kind="ExternalInput").ap()
    acb = nc.dram_tensor("acb", [D, 1], f32, kind="ExternalInput").ap()
    rbeff = nc.dram_tensor("rbeff", [1, E], f32, kind="ExternalInput").ap()
    eye = nc.dram_tensor("eye", [128, 128], f32, kind="ExternalInput").ap()
    eyehi = nc.dram_tensor("eyehi", [128, 64], f32, kind="ExternalInput").ap()
    offt = nc.dram_tensor("offt", [128, BLOC], i32, kind="ExternalInput").ap()
    offc = nc.dram_tensor("offc", [128, BLOC // 2], i32, kind="ExternalInput").ap()
    maskT = nc.dram_tensor("maskT", [M, BLOC], f32, kind="ExternalInput").ap()
    out = nc.dram_tensor("out", [BLOC, J, E], f32, kind="ExternalOutput").ap()

    AF = mybir.ActivationFunctionType

    with tile.TileContext(nc) as tc, ExitStack() as ctx:
        const = ctx.enter_context(tc.tile_pool(name="const", bufs=1))
        vgp = ctx.enter_context(tc.tile_pool(name="vg", bufs=BLOC // 4))
        ugp = ctx.enter_context(tc.tile_pool(name="ug", bufs=BLOC // 8))
        work = ctx.enter_context(tc.tile_pool(name="work", bufs=5))
        vecp = ctx.enter_context(tc.tile_pool(name="vec", bufs=6))
        outp = ctx.enter_context(tc.tile_pool(name="outp", bufs=3))
        psb = ctx.enter_context(tc.tile_pool(name="psb", bufs=3, space="PSUM"))
        psd = ctx.enter_context(tc.tile_pool(name="psd", bufs=3, space="PSUM"))
        psv = ctx.enter_context(tc.tile_pool(name="psv", bufs=2, space="PSUM"))

        # --- constants ---
        eye_t = const.tile([128, 128], f32)
        nc.sync.dma_start(out=eye_t[:], in_=eye[:, :])
        eyehi_t = const.tile([128, 64], f32)
        nc.sync.dma_start(out=eyehi_t[:], in_=eyehi[:, :])
        atwT_t = const.tile([E, D], f32)
        nc.sync.dma_start(out=atwT_t[:], in_=atwT[:, :])
        acwT_t = const.tile([E, D], f32)
        nc.sync.dma_start(out=acwT_t[:], in_=acwT[:, :])
        bcwT_t = const.tile([E, E], f32)
        nc.sync.dma_start(out=bcwT_t[:], in_=bcwT[:, :])
        rwT_t = const.tile([E, E], f32)
        nc.sync.dma_start(out=rwT_t[:], in_=rwT[:, :])
        atb_t = const.tile([D, 1], f32)
        nc.sync.dma_start(out=atb_t[:], in_=atb[:, :])
        acb_t = const.tile([D, 1], f32)
        nc.sync.dma_start(out=acb_t[:], in_=acb[:, :])
        rb_t = const.tile([128, E], f32)
        rb_bcast = bass.AP(tensor=rbeff.tensor, offset=0, ap=[[0, 128], [1, E]])
        nc.sync.dma_start(out=rb_t[:], in_=rb_bcast)
        offt_t = const.tile([128, BLOC], i32)
        nc.sync.dma_start(out=offt_t[:], in_=offt[:, :])
        offc_t = const.tile([128, BLOC // 2], i32)
        nc.sync.dma_start(out=offc_t[:], in_=offc[:, :])
        maskT_t = const.tile([M, BLOC], f32)
        nc.sync.dma_start(out=maskT_t[:], in_=maskT[:, :])
        ones_t = const.tile([128, 1], f32)
        nc.vector.memset(ones_t[:], 1.0)
        eps_t = const.tile([128, 1], f32)
        nc.vector.memset(eps_t[:], EPS2)

        # --- gathers: 4 blocks of 128 rows per indirect DMA instruction.
        # batch b's 101 target rows = partitions 0..100 of v block b;
        # batch b's 50 context rows = partitions 64*(b%2).. of u block b//2
        vg = [None] * (BLOC // 4)
        ug = [None] * (BLOC // 8)
        for t in range(BLOC // 8):
            for qq in (2 * t, 2 * t + 1):
                g = vgp.tile([128, 4, E], f32, tag="vg")
                for j in range(4):
                    nc.gpsimd.indirect_dma_start(
                        out=g[:, j, :],
                        out_offset=None,
                        in_=temb[:, :],
                        in_offset=bass.IndirectOffsetOnAxis(
                            ap=offt_t[:, 4 * qq + j : 4 * qq + j + 1], axis=0
                        ),
                    )
                vg[qq] = g
            g = ugp.tile([128, 4, E], f32, tag="ug")
            for j in range(4):
                nc.gpsimd.indirect_dma_start(
                    out=g[:, j, :],
                    out_offset=None,
                    in_=cemb[:, :],
                    in_offset=bass.IndirectOffsetOnAxis(
                        ap=offc_t[:, 4 * t + j : 4 * t + j + 1], axis=0
                    ),
                )
            ug[t] = g

        # --- per-batch compute, stage1/stage2 pipelined emission ---
        def stage1(b):
            v_ap = vg[b // 4][:J, b % 4, :]  # [101,128]
            ublk = b // 2
            uo = 64 * (b % 2)
            u_ap = ug[ublk // 4][uo : uo + M, ublk % 4, :]  # [50,128]

            # transposes to feature-major
            vT_ps = psb.tile([128, 128], f32, tag="pbig", space="PSUM")
            nc.tensor.transpose(out=vT_ps[:, :J], in_=v_ap, identity=eye_t[:J, :J])
            vT = work.tile([E, J], f32, tag="vT")
            nc.vector.tensor_copy(out=vT[:], in_=vT_ps[:, :J])

            uT_ps = psb.tile([128, 128], f32, tag="pbig", space="PSUM")
            u_ident = eye_t[:M, :M] if uo == 0 else eyehi_t[uo : uo + M, :M]
            nc.tensor.transpose(out=uT_ps[:, :M], in_=u_ap, identity=u_ident)
            uT = work.tile([E, M], f32, tag="uT")
            nc.scalar.copy(out=uT[:], in_=uT_ps[:, :M])

            # projections (feature-major), bias added during PSUM->SBUF copy
            tpT_ps = psb.tile([128, 128], f32, tag="pbig", space="PSUM")
            nc.tensor.matmul(
                out=tpT_ps[:D, :J], lhsT=atwT_t[:], rhs=vT[:], start=True, stop=True
            )
            tpT = work.tile([D, J], f32, tag="tpT")
            nc.scalar.activation(
                out=tpT[:], in_=tpT_ps[:D, :J], func=AF.Identity, bias=atb_t[:], scale=1.0
            )

            cpT_ps = psb.tile([128, 128], f32, tag="pbig", space="PSUM")
            nc.tensor.matmul(
                out=cpT_ps[:D, :M], lhsT=acwT_t[:], rhs=uT[:], start=True, stop=True
            )
            cpT = work.tile([D, M], f32, tag="cpT")
            nc.scalar.activation(
                out=cpT[:], in_=cpT_ps[:D, :M], func=AF.Identity, bias=acb_t[:], scale=1.0
            )

            # squared norms via matmul-with-ones -> column vectors
            tpT2 = work.tile([D, J], f32, tag="tpT2")
            nc.vector.tensor_mul(out=tpT2[:], in0=tpT[:], in1=tpT[:])
            cpT2 = work.tile([D, M], f32, tag="cpT2")
            nc.vector.tensor_mul(out=cpT2[:], in0=cpT[:], in1=cpT[:])

            nt2_ps = psv.tile([128, 1], f32, tag="pvec", space="PSUM")
            nc.tensor.matmul(
                out=nt2_ps[:J, :], lhsT=tpT2[:], rhs=ones_t[:D, :], start=True, stop=True
            )
            nc2_ps = psv.tile([128, 1], f32, tag="pvec", space="PSUM")
            nc.tensor.matmul(
                out=nc2_ps[:M, :], lhsT=cpT2[:], rhs=ones_t[:D, :], start=True, stop=True
            )

            # 1/|x| = exp(-0.5 * ln(x^2 + eps)) -- Ln and Exp share one table
            lnt = vecp.tile([128, 1], f32, tag="lnt")
            nc.scalar.activation(
                out=lnt[:J], in_=nt2_ps[:J, :], func=AF.Ln, bias=eps_t[:J, :]
            )
            ntinv = vecp.tile([128, 1], f32, tag="ntinv")
            nc.scalar.activation(
                out=ntinv[:J], in_=lnt[:J], func=AF.Exp, scale=-0.5
            )

            lnc = vecp.tile([128, 1], f32, tag="lnc")
            nc.scalar.activation(
                out=lnc[:M], in_=nc2_ps[:M, :], func=AF.Ln, bias=eps_t[:M, :]
            )
            ncinv = vecp.tile([128, 1], f32, tag="ncinv")
            nc.scalar.activation(
                out=ncinv[:M], in_=lnc[:M], func=AF.Exp, scale=-0.5
            )

            # dot products (own double-buffered bank; spans into stage2)
            dot_ps = psd.tile([128, 128], f32, tag="pdot", space="PSUM")
            nc.tensor.matmul(
                out=dot_ps[:J, :M], lhsT=tpT[:], rhs=cpT[:], start=True, stop=True
            )
            return dot_ps, ntinv, ncinv, uT

        def stage2(b, st):
            dot_ps, ntinv, ncinv, uT = st
            dotn = work.tile([J, M], f32, tag="dotn")
            nc.vector.tensor_scalar_mul(dotn[:], dot_ps[:J, :M], ntinv[:J, :])

            # transpose to [50,101]; exp(ncinv*x + mask) in one activation
            dotT_ps = psb.tile([128, 128], f32, tag="pbig", space="PSUM")
            nc.tensor.transpose(
                out=dotT_ps[:M, :J], in_=dotn[:], identity=eye_t[:J, :J]
            )
            attnT = work.tile([M, J], f32, tag="attnT")
            nc.scalar.activation(
                out=attnT[:],
                in_=dotT_ps[:M, :J],
                func=AF.Exp,
                bias=maskT_t[:, b : b + 1],
                scale=ncinv[:M, :],
            )

            # softmax denominators (per target row j)
            cs_ps = psv.tile([128, 1], f32, tag="pvec", space="PSUM")
            nc.tensor.matmul(
                out=cs_ps[:J, :], lhsT=attnT[:], rhs=ones_t[:M, :], start=True, stop=True
            )
            rsinv = vecp.tile([128, 1], f32, tag="rsinv")
            nc.vector.reciprocal(out=rsinv[:J], in_=cs_ps[:J, :1])


            # bu = u @ Bc_w.T (bias folded into rbeff), then alphaT, then output

            bu_ps = psb.tile([128, 128], f32, tag="pbig", space="PSUM")
            nc.tensor.matmul(
                out=bu_ps[:M, :E], lhsT=uT[:], rhs=bcwT_t[:], start=True, stop=True
            )
            bu = work.tile([M, E], f32, tag="bu")
            nc.scalar.copy(out=bu[:], in_=bu_ps[:M, :])

            al_ps = psb.tile([128, 128], f32, tag="pbig", space="PSUM")
            nc.tensor.matmul(
                out=al_ps[:E, :J], lhsT=bu[:], rhs=attnT[:], start=True, stop=True
            )
            alT = work.tile([E, J], f32, tag="alT")
            nc.vector.tensor_copy(out=alT[:], in_=al_ps[:, :J])

            o_ps = psb.tile([128, 128], f32, tag="pbig", space="PSUM")
            nc.tensor.matmul(
                out=o_ps[:J, :E], lhsT=alT[:], rhs=rwT_t[:], start=True, stop=True
            )
            o_sb = outp.tile([J, E], f32, tag="o")
            nc.vector.scalar_tensor_tensor(
                out=o_sb[:], in0=o_ps[:J, :E], scalar=rsinv[:J, :],
                in1=rb_t[:J, :], op0=mybir.AluOpType.mult,
                op1=mybir.AluOpType.add,
            )
            nc.sync.dma_start(out=out[b], in_=o_sb[:])

        # per-engine execution is in program order: interleave batch b+1's
        # stage1 with batch b's stage2 so independent work hides the waits
        LAG = 2
        pend = [(0, stage1(0))]
        for b in range(1, BLOC):
            pend.append((b, stage1(b)))
            if len(pend) > LAG:
                stage2(*pend.pop(0))
        while pend:
            stage2(*pend.pop(0))

    # Force every activation onto the one table holding Ln/Exp/Copy/Identity
    # so the kernel pays a single table load. Indices into act_info.json are
    # preserved (other sets are just emptied for the placement pass), so the
    # runtime table mapping stays correct.
    if NOPATCH:
        nc.compile()
    else:
        orig = bacc_mod.get_activation_tables
        bacc_mod.get_activation_tables = _patched_tables(orig)
        try:
            nc.compile()
        finally:
            bacc_mod.get_activation_tables = orig
    return nc


def _get_program():
    if "nc" not in _CACHE:
        _CACHE["nc"] = _build_program()
    return _CACHE["nc"]


def _prep_inputs(batch_titems, batch_citems, batch_pad_ids, t_emb, c_emb,
                 Ac_w, Ac_b, At_w, At_b, Bc_w, Bc_b, R_w, R_b):
    f = lambda x: np.ascontiguousarray(np.asarray(x, dtype=np.float32))
    t_emb = f(t_emb)
    c_emb = f(c_emb)
    tit = np.asarray(batch_titems).astype(np.int32)
    cit = np.asarray(batch_citems).astype(np.int32)
    pad = np.asarray(batch_pad_ids).astype(np.int64)

    mask = np.zeros((B, M), np.float32)
    mask[pad[0], pad[1]] = NEG

    atwT = f(np.asarray(At_w).T)
    acwT = f(np.asarray(Ac_w).T)
    bcwT = f(np.asarray(Bc_w).T)
    rwT = f(np.asarray(R_w).T)
    atb = f(np.asarray(At_b).reshape(D, 1))
    acb = f(np.asarray(Ac_b).reshape(D, 1))
    rbeff = f(
        (np.asarray(R_b, np.float32)
         + np.asarray(R_w, np.float32) @ np.asarray(Bc_b, np.float32)).reshape(1, E)
    )
    eye = np.eye(128, dtype=np.float32)
    eyehi = np.zeros((128, 64), np.float32)
    eyehi[64:, :] = np.eye(64, dtype=np.float32)

    in_maps = []
    for c in range(NCORES):
        s = c * BLOC
        tslice = tit[s : s + BLOC]  # [64,101]
        tpad = np.zeros((BLOC, 128), np.int32)
        tpad[:, :J] = tslice
        offt = np.ascontiguousarray(tpad.reshape(-1).reshape(BLOC, 128).T)

        cslice = cit[s : s + BLOC]  # [64,50]
        cpad = np.zeros((BLOC, 64), np.int32)
        cpad[:, :M] = cslice
        offc = np.ascontiguousarray(cpad.reshape(-1).reshape(BLOC // 2, 128).T)

        maskTc = np.ascontiguousarray(mask[s : s + BLOC].T)  # [50,64]

        in_maps.append(
            {
                "t_emb": t_emb,
                "c_emb": c_emb,
                "atwT": atwT,
                "acwT": acwT,
                "bcwT": bcwT,
                "rwT": rwT,
                "atb": atb,
                "acb": acb,
                "rbeff": rbeff,
                "eye": eye,
                "eyehi": eyehi,
                "offt": offt,
                "offc": offc,
                "maskT": maskTc,
            }
        )
    return in_maps


def run_sharded(in_maps, **kwargs):
    from concourse.bass_utils import run_bass_kernel_spmd

    nc = _get_program()
    res = run_bass_kernel_spmd(nc, in_maps, core_ids=list(range(NCORES)), **kwargs)
    outs = [res.results[c]["out"] for c in range(NCORES)]
    full = np.concatenate(outs, axis=0)
    return full, res


def kernel(**inputs):
    in_maps = _prep_inputs(**inputs)
    full, _ = run_sharded(in_maps)
    return full.astype(np.float32)



# revision 12
# speedup vs baseline: 2.6465x; 2.6465x over previous
"""AttentiveItemToVec Trainium2 kernel (batched bf16 redesign).

Full-input contract: kernel(**inputs) takes the unsharded numpy inputs and
returns the full [512, 101, 128] float32 output. Internally shards the batch
across 8 NeuronCores (64 batches each), runs a Bass/Tile kernel per core via
run_bass_kernel_spmd, and concatenates the per-core outputs.

Key structure (per core, 64 batches):
- Embeddings are uploaded as bf16 and fetched with 12 large multi-block
  indirect DMAs (1024 rows each), amortizing the ~1us SWDGE fixed cost.
- R_w is folded into Bc_w on the host (W = (R_w @ Bc_w)^T), removing the
  entire output projection stage; the combined bias rbeff = R_b + R_w@Bc_b
  is added in the final scale-and-bias op.
- PE transposes write 8 batches into one PSUM bank; a single DVE 2x copy
  moves each group to SBUF. Projections run in 404/512-column chunks with
  the bias folded into the PSUM->SBUF copy (round-robined over DVE/Act/Pool).
- Norm sums land in shared PSUM tiles (one column per batch/block) so the
  1/|x| = exp(-0.5*ln(x^2+eps)) pipeline is 4 batched activations total.
- Per-batch chain: dot matmul -> DVE row scale (ntinv) -> PE transpose ->
  Act exp (scale=ncinv, bias=pad mask) -> cs matmul into a shared column.
  Reciprocals run per 16-batch group; the output stage (attn @ bu, then
  gpsimd scale+bias) is interleaved one group behind, with 8-batch output
  DMAs.
"""

import numpy as np
from contextlib import ExitStack

# Problem constants (hardcoded per contract).
V, E, D = 100000, 128, 60
B, J, M, P = 512, 101, 50, 5120
NCORES = 8
BLOC = B // NCORES  # 64 batches per core
NBLK_U = BLOC // 2  # 32 u blocks (2 batches each)
NEG = -1.0e30
EPS2 = 1e-12  # clamp on squared norms (eps=1e-6 on norms)

_CACHE = {}

_ACT_TABLE = "natural_log_exp_and_others"


def _patched_tables(orig_fn):
    def fn(arch):
        tabs = orig_fn(arch)
        return {
            name: (s if name == _ACT_TABLE else type(s)())
            for name, s in tabs.items()
        }
    return fn


def _build_program():
    import os
    NOPATCH = os.environ.get("K_NOPATCH") == "1"
    import concourse.bass as bass
    import concourse.tile as tile
    import concourse.bacc as bacc_mod
    from concourse import bacc, mybir

    f32 = mybir.dt.float32
    bf16 = mybir.dt.bfloat16
    i32 = mybir.dt.int32

    nc = bacc.Bacc(
        "TRN2",
        target_bir_lowering=False,
        debug=False,
        enable_asserts=False,
    )

    temb = nc.dram_tensor("t_emb", [V, E], bf16, kind="ExternalInput").ap()
    cemb = nc.dram_tensor("c_emb", [V, E], bf16, kind="ExternalInput").ap()
    atw = nc.dram_tensor("atw", [E, D], bf16, kind="ExternalInput").ap()
    acw = nc.dram_tensor("acw", [E, D], bf16, kind="ExternalInput").ap()
    wrb = nc.dram_tensor("wrb", [E, E], bf16, kind="ExternalInput").ap()
    atb = nc.dram_tensor("atb", [D, 1], f32, kind="ExternalInput").ap()
    acb = nc.dram_tensor("acb", [D, 1], f32, kind="ExternalInput").ap()
    rbeff = nc.dram_tensor("rbeff", [1, E], f32, kind="ExternalInput").ap()
    eye = nc.dram_tensor("eye", [128, 128], bf16, kind="ExternalInput").ap()
    offt = nc.dram_tensor("offt", [128, BLOC], i32, kind="ExternalInput").ap()
    offc = nc.dram_tensor("offc", [128, NBLK_U], i32, kind="ExternalInput").ap()
    maskT = nc.dram_tensor("maskT", [M, BLOC], f32, kind="ExternalInput").ap()
    out = nc.dram_tensor("out", [BLOC, J, E], f32, kind="ExternalOutput").ap()

    AF = mybir.ActivationFunctionType
    ALU = mybir.AluOpType

    with tile.TileContext(nc) as tc, ExitStack() as ctx:
        const = ctx.enter_context(tc.tile_pool(name="const", bufs=1))
        big = ctx.enter_context(tc.tile_pool(name="big", bufs=1))
        outp = ctx.enter_context(tc.tile_pool(name="outp", bufs=2))
        dotp = ctx.enter_context(tc.tile_pool(name="dotp", bufs=3))

        # --- constants ---
        eye_t = const.tile([128, 128], bf16)
        nc.sync.dma_start(out=eye_t[:], in_=eye[:, :])
        atw_t = const.tile([E, D], bf16)
        nc.sync.dma_start(out=atw_t[:], in_=atw[:, :])
        acw_t = const.tile([E, D], bf16)
        nc.sync.dma_start(out=acw_t[:], in_=acw[:, :])
        wrb_t = const.tile([E, E], bf16)
        nc.sync.dma_start(out=wrb_t[:], in_=wrb[:, :])
        atb_t = const.tile([D, 1], f32)
        nc.sync.dma_start(out=atb_t[:], in_=atb[:, :])
        acb_t = const.tile([D, 1], f32)
        nc.sync.dma_start(out=acb_t[:], in_=acb[:, :])
        rb_t = const.tile([128, E], f32)
        rb_bcast = bass.AP(tensor=rbeff.tensor, offset=0, ap=[[0, 128], [1, E]])
        nc.sync.dma_start(out=rb_t[:], in_=rb_bcast)
        offt_t = const.tile([128, BLOC], i32)
        nc.sync.dma_start(out=offt_t[:], in_=offt[:, :])
        offc_t = const.tile([128, NBLK_U], i32)
        nc.sync.dma_start(out=offc_t[:], in_=offc[:, :])
        maskT_t = const.tile([M, BLOC], f32)
        nc.sync.dma_start(out=maskT_t[:], in_=maskT[:, :])
        ones_t = const.tile([128, 1], bf16)
        nc.vector.memset(ones_t[:], 1.0)
        eps_t = const.tile([128, 1], f32)
        nc.vector.memset(eps_t[:], EPS2)

        # --- big SBUF arrays ---
        vg = big.tile([128, BLOC, E], bf16)        # gathered t_emb rows
        ug = big.tile([128, NBLK_U, E], bf16)      # gathered c_emb rows
        vT_all = big.tile([128, BLOC * J], bf16)   # feature-major v
        uT_all = big.tile([128, NBLK_U * 128], bf16)
        tpT_all = big.tile([D, BLOC * J], bf16)    # projected targets (+bias)
        cpT_all = big.tile([D, NBLK_U * 128], bf16)
        tp2_all = big.tile([D, BLOC * J], bf16)    # squares
        cp2_all = big.tile([D, NBLK_U * 128], bf16)
        bu_all = big.tile([M, BLOC * E], bf16)     # u @ (R Bc)^T, item-major
        attnT = big.tile([M, BLOC * J], bf16)      # exp'd scores [m, j] per batch
        ntinv = big.tile([128, BLOC], f32)
        ncinv = big.tile([M, BLOC], f32)
        lnt_t = big.tile([128, BLOC], f32)
        lnc_t = big.tile([M, BLOC], f32)
        rs_all = big.tile([128, BLOC], f32)        # 1/softmax-denominator

        # --- gathers: 1024 rows per indirect DMA (8 blocks of 128) ---
        GV, GU = 8, 8  # blocks per gather instruction
        for g in range(BLOC // GV):
            nc.gpsimd.indirect_dma_start(
                out=vg[:, GV * g : GV * (g + 1), :],
                out_offset=None,
                in_=temb[:, :],
                in_offset=bass.IndirectOffsetOnAxis(
                    ap=offt_t[:, GV * g : GV * (g + 1)], axis=0
                ),
            )
        for h in range(NBLK_U // GU):
            nc.gpsimd.indirect_dma_start(
                out=ug[:, GU * h : GU * (h + 1), :],
                out_offset=None,
                in_=cemb[:, :],
                in_offset=bass.IndirectOffsetOnAxis(
                    ap=offc_t[:, GU * h : GU * (h + 1)], axis=0
                ),
            )

        with tc.tile_pool(name="ps_tr", bufs=2, space="PSUM") as ps_tr, \
             tc.tile_pool(name="ps_pj", bufs=2, space="PSUM") as ps_pj, \
             tc.tile_pool(name="ps_bu", bufs=2, space="PSUM") as ps_bu, \
             tc.tile_pool(name="ps_nrm", bufs=1, space="PSUM") as ps_nrm:

            # --- phase 1: transposes, 8 batches/blocks per PSUM bank ---
            for g in range(BLOC // 8):  # v: groups of 8 batches
                tr = ps_tr.tile([128, 1024], bf16, tag="tr", space="PSUM")
                for i in range(8):
                    b = 8 * g + i
                    nc.tensor.transpose(
                        out=tr[:, J * i : J * (i + 1)],
                        in_=vg[:, b, :],
                        identity=eye_t[:, :J],
                    )
                nc.vector.tensor_copy(
                    out=vT_all[:, 8 * J * g : 8 * J * (g + 1)],
                    in_=tr[:, : 8 * J],
                )
            for h in range(NBLK_U // 8):  # u: groups of 8 blocks
                tr = ps_tr.tile([128, 1024], bf16, tag="tr", space="PSUM")
                for i in range(8):
                    t = 8 * h + i
                    nc.tensor.transpose(
                        out=tr[:, 128 * i : 128 * (i + 1)],
                        in_=ug[:, t, :],
                        identity=eye_t[:, :],
                    )
                nc.vector.tensor_copy(
                    out=uT_all[:, 1024 * h : 1024 * (h + 1)],
                    in_=tr[:, :],
                )

            # --- phase 2: projections (+bias in copy), squares, bu ---
            def bias_copy(k, dst, src, bias_t):
                # round-robin the PSUM->SBUF bias-add over DVE / Act / Pool
                eng = k % 3
                if eng == 0:
                    nc.vector.tensor_scalar_add(out=dst, in0=src, scalar1=bias_t)
                elif eng == 1:
                    nc.scalar.activation(
                        out=dst, in_=src, func=AF.Identity, bias=bias_t, scale=1.0
                    )
                else:
                    nc.gpsimd.tensor_scalar_add(out=dst, in0=src, scalar1=bias_t)

            CV = 4 * J  # 404 cols (4 batches) per v-chunk
            k = 0
            for c in range(BLOC // 4):
                pj = ps_pj.tile([D, 512], f32, tag="pj", space="PSUM")
                s = CV * c
                nc.tensor.matmul(
                    out=pj[:, :CV], lhsT=atw_t[:], rhs=vT_all[:, s : s + CV],
                    start=True, stop=True,
                )
                bias_copy(k, tpT_all[:, s : s + CV], pj[:, :CV], atb_t[:])
                k += 1
                nc.vector.tensor_mul(
                    out=tp2_all[:, s : s + CV],
                    in0=tpT_all[:, s : s + CV],
                    in1=tpT_all[:, s : s + CV],
                )
            for c in range(NBLK_U // 4):
                pj = ps_pj.tile([D, 512], f32, tag="pj", space="PSUM")
                s = 512 * c
                nc.tensor.matmul(
                    out=pj[:, :], lhsT=acw_t[:], rhs=uT_all[:, s : s + 512],
                    start=True, stop=True,
                )
                bias_copy(k, cpT_all[:, s : s + 512], pj[:, :], acb_t[:])
                k += 1
                nc.vector.tensor_mul(
                    out=cp2_all[:, s : s + 512],
                    in0=cpT_all[:, s : s + 512],
                    in1=cpT_all[:, s : s + 512],
                )
            def ucol(b):
                return 128 * (b // 2) + 64 * (b % 2)

            for c in range(BLOC // 4):  # bu: 4 batches per PSUM bank
                bu_ps = ps_bu.tile([M, 512], f32, tag="bu", space="PSUM")
                for i in range(4):
                    b = 4 * c + i
                    nc.tensor.matmul(
                        out=bu_ps[:, 128 * i : 128 * (i + 1)],
                        lhsT=uT_all[:, ucol(b) : ucol(b) + M],
                        rhs=wrb_t[:],
                        start=True, stop=True,
                    )
                eng = (nc.vector, nc.scalar, nc.gpsimd)[c % 3]
                if eng is nc.scalar:
                    nc.scalar.copy(
                        out=bu_all[:, 512 * c : 512 * (c + 1)], in_=bu_ps[:, :]
                    )
                else:
                    eng.tensor_copy(
                        out=bu_all[:, 512 * c : 512 * (c + 1)], in_=bu_ps[:, :]
                    )

            # --- phase 3: norms (batched Ln/Exp over shared PSUM columns) ---
            nt_ps = ps_nrm.tile([128, BLOC], f32, tag="nt", space="PSUM")
            nc_ps = ps_nrm.tile([M, BLOC], f32, tag="ncn", space="PSUM")
            for b in range(BLOC):
                nc.tensor.matmul(
                    out=nt_ps[:J, b : b + 1],
                    lhsT=tp2_all[:, J * b : J * (b + 1)],
                    rhs=ones_t[:D, :],
                    start=True, stop=True,
                )
            for b in range(BLOC):
                nc.tensor.matmul(
                    out=nc_ps[:, b : b + 1],
                    lhsT=cp2_all[:, ucol(b) : ucol(b) + M],
                    rhs=ones_t[:D, :],
                    start=True, stop=True,
                )
            nc.scalar.activation(
                out=lnt_t[:], in_=nt_ps[:, :], func=AF.Ln, bias=eps_t[:]
            )
            nc.scalar.activation(
                out=ntinv[:], in_=lnt_t[:], func=AF.Exp, scale=-0.5
            )
            nc.scalar.activation(
                out=lnc_t[:], in_=nc_ps[:, :], func=AF.Ln, bias=eps_t[:M]
            )
            nc.scalar.activation(
                out=ncinv[:], in_=lnc_t[:], func=AF.Exp, scale=-0.5
            )

        # --- phase 4: per-batch attention, pipelined in groups of 16 ---
        with tc.tile_pool(name="ps_dot", bufs=2, space="PSUM") as ps_dot, \
             tc.tile_pool(name="ps_dT", bufs=2, space="PSUM") as ps_dT, \
             tc.tile_pool(name="ps_cs", bufs=1, space="PSUM") as ps_cs, \
             tc.tile_pool(name="ps_o", bufs=3, space="PSUM") as ps_o:

            cs_ps = ps_cs.tile([128, BLOC], f32, tag="cs", space="PSUM")
            GRP = 16

            def stageA(b):
                s = J * b
                dot_ps = ps_dot.tile([J, 64], f32, tag="dot", space="PSUM")
                nc.tensor.matmul(
                    out=dot_ps[:, :M],
                    lhsT=tpT_all[:, s : s + J],
                    rhs=cpT_all[:, ucol(b) : ucol(b) + M],
                    start=True, stop=True,
                )
                dotn = dotp.tile([J, M], bf16, tag="dotn")
                nc.vector.tensor_scalar_mul(
                    dotn[:], dot_ps[:, :M], ntinv[:J, b : b + 1]
                )
                dT_ps = ps_dT.tile([M, 128], bf16, tag="dT", space="PSUM")
                nc.tensor.transpose(
                    out=dT_ps[:, :J], in_=dotn[:], identity=eye_t[:J, :J]
                )
                nc.scalar.activation(
                    out=attnT[:, s : s + J],
                    in_=dT_ps[:, :J],
                    func=AF.Exp,
                    bias=maskT_t[:, b : b + 1],
                    scale=ncinv[:, b : b + 1],
                )

            def stageCS(b):
                s = J * b
                nc.tensor.matmul(
                    out=cs_ps[:J, b : b + 1],
                    lhsT=attnT[:, s : s + J],
                    rhs=ones_t[:M, :],
                    start=True, stop=True,
                )

            def stageB(b, obuf):
                s = J * b
                o_ps = ps_o.tile([J, E], f32, tag="o", space="PSUM")
                nc.tensor.matmul(
                    out=o_ps[:, :],
                    lhsT=attnT[:, s : s + J],
                    rhs=bu_all[:, E * b : E * (b + 1)],
                    start=True, stop=True,
                )
                k = b % 8
                eng = nc.gpsimd if b % 2 == 0 else nc.vector
                eng.scalar_tensor_tensor(
                    out=obuf[:, E * k : E * (k + 1)],
                    in0=o_ps[:, :],
                    scalar=rs_all[:J, b : b + 1],
                    in1=rb_t[:J, :],
                    op0=ALU.mult,
                    op1=ALU.add,
                )

            def flush(b0, obuf):
                # batches [b0, b0+8) -> out[b0:b0+8] in one DMA
                dst = bass.AP(
                    tensor=out.tensor,
                    offset=b0 * J * E,
                    ap=[[E, J], [J * E, 8], [1, E]],
                )
                nc.sync.dma_start(out=dst, in_=obuf[:, :])

            ngrp = BLOC // GRP
            pendB = []  # batches awaiting stage B
            obuf = None
            nflush = 0

            def runB():
                nonlocal obuf, nflush
                b = pendB.pop(0)
                if obuf is None:
                    obuf = outp.tile([J, 8 * E], f32, tag="ob")
                stageB(b, obuf)
                if b % 8 == 7:
                    flush(b - 7, obuf)
                    obuf = None
                    nflush += 1

            for g in range(ngrp):
                for i in range(GRP):
                    b = GRP * g + i
                    stageA(b)
                    if b > 0:
                        stageCS(b - 1)
                    if pendB:
                        runB()
                stageCS(GRP * g + GRP - 1)
                nc.vector.reciprocal(
                    out=rs_all[:, GRP * g : GRP * (g + 1)],
                    in_=cs_ps[:, GRP * g : GRP * (g + 1)],
                )
                pendB.extend(range(GRP * g, GRP * (g + 1)))
            while pendB:
                runB()

    # Force every activation onto the one table holding Ln/Exp/Copy/Identity
    # so the kernel pays a single table load.
    if NOPATCH:
        nc.compile()
    else:
        orig = bacc_mod.get_activation_tables
        bacc_mod.get_activation_tables = _patched_tables(orig)
        try:
            nc.compile()
        finally:
            bacc_mod.get_activation_tables = orig
    return nc


def _get_program():
    if "nc" not in _CACHE:
        _CACHE["nc"] = _build_program()
    return _CACHE["nc"]


def _prep_inputs(batch_titems, batch_citems, batch_pad_ids, t_emb, c_emb,
                 Ac_w, Ac_b, At_w, At_b, Bc_w, Bc_b, R_w, R_b):
    import ml_dtypes
    bf16 = ml_dtypes.bfloat16

    f = lambda x: np.ascontiguousarray(np.asarray(x, dtype=np.float32))
    fb = lambda x: np.ascontiguousarray(np.asarray(x, dtype=np.float32).astype(bf16))
    t_emb_b = fb(t_emb)
    c_emb_b = fb(c_emb)
    tit = np.asarray(batch_titems).astype(np.int32)
    cit = np.asarray(batch_citems).astype(np.int32)
    pad = np.asarray(batch_pad_ids).astype(np.int64)

    mask = np.zeros((B, M), np.float32)
    mask[pad[0], pad[1]] = NEG

    R_wf = np.asarray(R_w, np.float32)
    Bc_wf = np.asarray(Bc_w, np.float32)
    atw = fb(np.asarray(At_w).T)
    acw = fb(np.asarray(Ac_w).T)
    wrb = fb((R_wf @ Bc_wf).T)
    atb = f(np.asarray(At_b).reshape(D, 1))
    acb = f(np.asarray(Ac_b).reshape(D, 1))
    rbeff = f(
        (np.asarray(R_b, np.float32) + R_wf @ np.asarray(Bc_b, np.float32)
         ).reshape(1, E)
    )
    eye = np.eye(128, dtype=np.float32).astype(bf16)

    in_maps = []
    for c in range(NCORES):
        s = c * BLOC
        offt = np.zeros((128, BLOC), np.int32)
        offt[:J, :] = tit[s : s + BLOC].T

        offc = np.zeros((128, NBLK_U), np.int32)
        cs = cit[s : s + BLOC]  # [64, 50]
        offc[:M, :] = cs[0::2].T
        offc[64 : 64 + M, :] = cs[1::2].T

        maskTc = np.ascontiguousarray(mask[s : s + BLOC].T)  # [50,64]

        in_maps.append(
            {
                "t_emb": t_emb_b,
                "c_emb": c_emb_b,
                "atw": atw,
                "acw": acw,
                "wrb": wrb,
                "atb": atb,
                "acb": acb,
                "rbeff": rbeff,
                "eye": eye,
                "offt": np.ascontiguousarray(offt),
                "offc": np.ascontiguousarray(offc),
                "maskT": maskTc,
            }
        )
    return in_maps


def run_sharded(in_maps, **kwargs):
    from concourse.bass_utils import run_bass_kernel_spmd

    nc = _get_program()
    res = run_bass_kernel_spmd(nc, in_maps, core_ids=list(range(NCORES)), **kwargs)
    outs = [res.results[c]["out"] for c in range(NCORES)]
    full = np.concatenate(outs, axis=0)
    return full, res


def kernel(**inputs):
    in_maps = _prep_inputs(**inputs)
    full, _ = run_sharded(in_maps)
    return full.astype(np.float32)
